# revision 18
# baseline (speedup 1.0000x reference)
"""GraphConv VAE encoder (3x GraphConv + reparameterization) on 8 Trainium2 cores.

Strategy (graph/data parallel, dst-sharded), v2:
  - Nodes padded to NPAD = 8*SH and sharded by dst across 8 cores.
  - Layer-1 projection hp = (feat @ W1) * ns computed on each core for its own
    node shard (host pre-transposes feat so no on-chip transposes are needed),
    then AllGather -> full bf16 gather table.
  - Edges are dst-sorted into 128-dst "sblocks", grouped 8 sblocks per
    supergroup, and split into 4 src-range buckets (dma_gather indices are
    int16, so gather tables are addressed in 4 windows of NPAD/4 rows).
  - Per 128-edge chunk: dma_gather the source rows (partition = edge), build a
    one-hot selection matrix S via iota==dstloc on DVE, and matmul into a
    per-sblock PSUM accumulator.  Segment-sum therefore runs on the tensor
    engine; each sblock is one PSUM->SBUF pass, no read-modify-write.
  - Key algebraic restructure vs v1: segment-sum commutes with the dense
    output projections, so layers 2/3 aggregate the 128-wide hs = relu(.)*ns
    ONCE (z2 = A @ hs) and apply W_mu / W_ls per-sblock AFTER aggregation.
    This removes the replicated full-table hp23 projection pass and halves
    the second gather's bytes (256B rows instead of 512B).
  - The second gather pass accumulates z2^T directly (matmul operands
    swapped: lhsT=rows, rhs=S), so W_mu/W_ls apply without any transposes;
    the final epilogue mu + noise * exp(log_sigma) runs feature-major and the
    host un-transposes the [128, sh] output shard.
"""

import sys

sys.path.insert(0, '/opt/trn_rl_repo')

import numpy as np
import ml_dtypes

import concourse.bass as bass
import concourse.bacc as bacc
import concourse.mybir as mybir
import concourse.tile as tile
from concourse import library_config
from concourse.tile_rust import add_dep_helper
from concourse.vector_clock import ScopedClock
from concourse.bass_utils import run_bass_kernel_spmd

BF16 = mybir.dt.bfloat16
F32 = mybir.dt.float32
NPBF16 = ml_dtypes.bfloat16

NC = 8          # cores
P = 128         # partitions / sblock width
SG = 8          # sblocks per supergroup (one PSUM bank per sblock)
NBUCK = 4       # src-range buckets (int16 gather index limit)
SB = 32         # chunks per S-build batch
PAD_DSTLOC = 256.0  # dstloc value for padded slots (never matches iota 0..127)


def _patch_tile_drain():
    """This walrus build rejects >1 sync-wait on the kernel-tail Drain; spread
    the waits across chained drains."""
    if getattr(tile.TileContext, "_drain_patched", False):
        return

    def patched(self, tick_clock, wait_clock):
        drain_inst = self.nc.sync.drain()
        wait_clock.add_sem_waits(drain_inst.ins,
                                 ScopedClock({None: tick_clock.global_clock}))
        si = drain_inst.ins.sync_info
        if si is not None and si.on_wait and len(si.on_wait) > 1:
            waits = list(si.on_wait)
            si.on_wait = waits[:1]
            for w in waits[1:]:
                d2 = self.nc.sync.drain()
                d2.ins.sync_info = mybir.SyncInfo(on_wait=[w], on_update=[])
        self.nc.all_engine_barrier()
        assert self.sems is not None
        popped = self.nc._tile_sem_poison_stack.pop()
        assert popped is self._sem_poison
        self.nc.clear_and_free_semaphores(list(self.sems.allocated().values()))
        self.nc.all_engine_barrier()

    tile.TileContext._drain_and_barrier = patched
    tile.TileContext._drain_patched = True


def _build_template(edges, n_nodes, npad):
    """Host-side edge preprocessing shared by both gather passes.

    Slots are packed at 16-slot granularity per (sblock, bucket) cell and
    concatenated within each (supergroup, bucket) dma_gather call, so 128-slot
    matmul chunks may span sblocks.  Each (chunk, sblock) incidence gets its
    own dstloc column (slots outside the sblock's run are PAD-masked), and the
    matmul accumulates that chunk's rows into the sblock's PSUM bank.

    Returns the SPMD-shared template (call table / incidence metadata) and
    the per-core slot data (int16 gather indices, per-incidence dstloc).
    """
    src = edges[0].astype(np.int64)
    dst = edges[1].astype(np.int64)
    sh = npad // NC          # nodes per core shard
    nsb = sh // P            # sblocks per core
    brows = npad // NBUCK    # rows per gather bucket
    n_sg = (nsb + SG - 1) // SG
    sgs = [list(range(g * SG, min((g + 1) * SG, nsb))) for g in range(n_sg)]

    core = dst // sh
    k = (dst % sh) // P
    b = src // brows
    # cell id: (core, sg, b, k) major->minor defines the stream order
    sg_of_k = k // SG
    cell = ((core * n_sg + sg_of_k) * NBUCK + b) * nsb + k
    n_cells = NC * n_sg * NBUCK * nsb
    cnt = np.bincount(cell, minlength=n_cells).reshape(NC, n_sg, NBUCK, nsb)

    # shared slot counts per (k, b): max over cores, rounded up to 16
    C16 = np.zeros((nsb, NBUCK), np.int64)
    for g, ks in enumerate(sgs):
        for kk in ks:
            for bb in range(NBUCK):
                mx = int(cnt[:, g, bb, kk].max())
                C16[kk, bb] = -(-mx // 16) * 16
    # every sblock needs at least one incidence so its PSUM chain starts
    for kk in range(nsb):
        if C16[kk].sum() == 0:
            C16[kk, 0] = 16

    # call table: one dma_gather per (g, b); cells packed back-to-back,
    # call padded to a 128 multiple
    calls = []               # (g, b, slot_off, num_idxs)
    cell_off = {}            # (g, b, kk) -> global slot offset
    pos = 0
    for g, ks in enumerate(sgs):
        for bb in range(NBUCK):
            call_off = pos
            for kk in ks:
                cell_off[(g, bb, kk)] = pos
                pos += int(C16[kk, bb])
            ni = -(-(pos - call_off) // P) * P
            pos = call_off + ni
            calls.append((g, bb, call_off, ni))
    total_slots = pos
    n_chunks = total_slots // P

    # incidence metadata in stream order:
    # (kk, g, bb, chunk_global, s0, s1, start, stop)
    incs = []
    first_of = {}
    last_of = {}
    for (g, bb, off, ni) in calls:
        for kk in sgs[g]:
            c0 = cell_off[(g, bb, kk)]
            c1 = c0 + int(C16[kk, bb])
            p0 = c0
            while p0 < c1:
                chg = p0 // P
                p1 = min(c1, (chg + 1) * P)
                ii = len(incs)
                incs.append([kk, g, bb, chg, p0 - chg * P, p1 - chg * P,
                             False, False])
                if kk not in first_of or first_of[kk] is None:
                    first_of[kk] = ii
                if first_of.get(kk) is None:
                    first_of[kk] = ii
                last_of[kk] = ii
                p0 = p1
        if bb == NBUCK - 1:
            # group ends: mark start/stop for its sblocks and reset
            for kk in sgs[g]:
                incs[first_of[kk]][6] = True
                incs[last_of[kk]][7] = True
                first_of[kk] = None
    n_inc = len(incs)
    incs = [tuple(x) for x in incs]

    # per-core slot data
    order = np.argsort(cell, kind='stable')
    cell_sorted = cell[order]
    # rank within cell
    cell_start = np.searchsorted(cell_sorted, np.arange(n_cells), side='left')
    rank = np.arange(len(order)) - cell_start[cell_sorted]
    # map cell -> slot offset (per its core's template)
    cell_to_off = np.zeros(n_cells, np.int64)
    for (g, bb, kk), off in cell_off.items():
        for c in range(NC):
            gcell = ((c * n_sg + g) * NBUCK + bb) * nsb + kk
            cell_to_off[gcell] = off
    slot = cell_to_off[cell_sorted] + rank

    idx_vals = np.zeros((NC, total_slots), np.int16)
    dl_vals = np.full((NC, total_slots), PAD_DSTLOC, np.float32)
    csrc = src[order] - b[order] * brows
    cdst = dst[order] % P
    ccore = core[order]
    idx_vals[ccore, slot] = csrc.astype(np.int16)
    dl_vals[ccore, slot] = cdst.astype(np.float32)

    # wrap indices per call: within a call, slot j -> [j%16, off//16 + j//16]
    ni16 = total_slots // 16
    idx16 = np.zeros((NC, 16, ni16), np.int16)
    for (_, _, off, ni) in calls:
        blk = idx_vals[:, off:off + ni].reshape(NC, ni // 16, 16)
        idx16[:, :, off // 16:(off + ni) // 16] = blk.transpose(0, 2, 1)
    idx16 = np.tile(idx16, (1, 8, 1))  # replicate to 128 partitions

    # dstloc per incidence column: [p, i] = dstloc of slot chunk*128+p if
    # p is inside the incidence's run, else PAD
    dl_sl = dl_vals.reshape(NC, n_chunks, P)     # [NC, chunk, p]
    dstloc = np.full((NC, P, n_inc), PAD_DSTLOC, np.float32)
    for ii, (kk, g, bb, chg, s0, s1, st, sp) in enumerate(incs):
        dstloc[:, s0:s1, ii] = dl_sl[:, chg, s0:s1]
    dstloc = dstloc.astype(NPBF16)

    tpl = dict(sh=sh, nsb=nsb, brows=brows, sgs=sgs, calls=calls,
               incs=incs, n_inc=n_inc, n_chunks=n_chunks,
               total_slots=total_slots, ni16=ni16)
    return tpl, idx16, dstloc


def _build(feat, edges, W1, b1, W_mu, b_mu, W_ls, b_ls, noise):
    import os
    skip = os.environ.get("K_SKIP", "")
    repeat = int(os.environ.get("K_REPEAT", "1"))
    n_qs = int(os.environ.get("K_QS", "4"))
    sb = int(os.environ.get("K_SB", str(SB)))
    N, IN = feat.shape
    OUT = W1.shape[1]
    F2 = 2 * OUT
    assert OUT == P
    npad = -(-N // (NC * P)) * NC * P        # multiple of 8*128
    # bucket rows must fit int16 and divide into 128-aligned shards
    while npad % (NBUCK * P) != 0:
        npad += NC * P
    sh = npad // NC
    brows = npad // NBUCK
    assert brows <= 32768
    nsb = sh // P
    kin = IN // P

    tpl, idx16, dstloc = _build_template(edges, N, npad)
    sgs, calls, incs = tpl['sgs'], tpl['calls'], tpl['incs']
    n_inc, ni16 = tpl['n_inc'], tpl['ni16']

    # ---- host-side numeric prep (degrees from the index arrays) ----
    deg_out = np.bincount(edges[0], minlength=npad).astype(np.float64)
    deg_in = np.bincount(edges[1], minlength=npad).astype(np.float64)
    ns = np.clip(deg_out, 1.0, None) ** -0.5
    nd = np.clip(deg_in, 1.0, None) ** -0.5
    ns[N:] = 0.0
    nd[N:] = 0.0
    ns = ns.astype(np.float32)
    nd = nd.astype(np.float32)

    featp = np.zeros((npad, IN), np.float32)
    featp[:N] = feat
    noisep = np.zeros((npad, OUT), np.float32)
    noisep[:N] = noise

    featb = featp.astype(NPBF16)
    W1b = np.ascontiguousarray(W1.astype(NPBF16))
    W23 = np.concatenate([W_mu, W_ls], axis=1)
    W23b = np.ascontiguousarray(W23.astype(NPBF16))
    # W1 as [128, kin, 128]: [p, kc, j] = W1[kc*128+p, j]
    W1sb = np.ascontiguousarray(W1b.reshape(kin, P, OUT).transpose(1, 0, 2))

    zero_b1 = not np.any(b1)
    zero_b23 = (not np.any(b_mu)) and (not np.any(b_ls))

    iota4 = np.tile(np.arange(P, dtype=np.float32), sb)[None, :].repeat(P, 0)
    iota4 = iota4.astype(NPBF16)                       # [128, sb*128]
    b1r = np.tile(b1[None, :].astype(np.float32), (P, 1))
    bmuT = np.ascontiguousarray(b_mu.astype(np.float32).reshape(P, 1))
    blsT = np.ascontiguousarray(b_ls.astype(np.float32).reshape(P, 1))

    in_maps = []
    for c in range(NC):
        rows = slice(c * sh, (c + 1) * sh)
        fsh = featb[rows]                               # [sh, IN]
        featT = np.ascontiguousarray(
            fsh.T.reshape(kin, P, sh).transpose(1, 0, 2).reshape(P, kin * sh))
        nsc = np.ascontiguousarray(
            ns[rows].reshape(nsb, P).T)                 # [128, nsb]
        ndc = np.ascontiguousarray(nd[rows].reshape(nsb, P).T)
        # nd replicated across partitions: [128, sh] (free dim = node in shard)
        ndb = np.ascontiguousarray(
            np.broadcast_to(nd[rows][None, :], (P, sh)).astype(np.float32))
        # noise transposed: [128 feat, sh]
        noT = np.ascontiguousarray(noisep[rows].T)      # [128, sh] f32
        in_maps.append({
            "featT": featT, "W1sb": W1sb.reshape(P, kin * OUT),
            "W23sb": W23b, "b1r": b1r, "bmuT": bmuT, "blsT": blsT,
            "nsc": nsc, "ndc": ndc, "ndb": ndb, "noiseT": noT,
            "iota4": iota4,
            "idx16": np.ascontiguousarray(idx16[c]),
            "dstloc": np.ascontiguousarray(dstloc[c]),
        })

    # ---------------- device program ----------------
    _patch_tile_drain()
    nc = bacc.Bacc('TRN2', target_bir_lowering=False, debug=False,
                   num_swdge_queues=n_qs)

    featT_d = nc.dram_tensor("featT", [P, kin * sh], BF16, kind="ExternalInput")
    W1_d = nc.dram_tensor("W1sb", [P, kin * OUT], BF16, kind="ExternalInput")
    W23_d = nc.dram_tensor("W23sb", [P, F2], BF16, kind="ExternalInput")
    b1_d = nc.dram_tensor("b1r", [P, OUT], F32, kind="ExternalInput")
    bmu_d = nc.dram_tensor("bmuT", [P, 1], F32, kind="ExternalInput")
    bls_d = nc.dram_tensor("blsT", [P, 1], F32, kind="ExternalInput")
    ns_d = nc.dram_tensor("nsc", [P, nsb], F32, kind="ExternalInput")
    nd_d = nc.dram_tensor("ndc", [P, nsb], F32, kind="ExternalInput")
    ndb_d = nc.dram_tensor("ndb", [P, sh], F32, kind="ExternalInput")
    noiseT_d = nc.dram_tensor("noiseT", [P, sh], F32, kind="ExternalInput")
    iota_d = nc.dram_tensor("iota4", [P, sb * P], BF16, kind="ExternalInput")
    idx_d = nc.dram_tensor("idx16", [P, ni16], mybir.dt.int16,
                           kind="ExternalInput")
    dl_d = nc.dram_tensor("dstloc", [P, n_inc], BF16, kind="ExternalInput")
    y_d = nc.dram_tensor("y", [P, sh], F32, kind="ExternalOutput")

    replica = [list(range(NC))]

    with tile.TileContext(nc) as tc:
        import contextlib
        with contextlib.ExitStack() as ctx:
            dram = ctx.enter_context(tc.tile_pool(name="dram", bufs=1,
                                                  space="DRAM"))
            cpool = ctx.enter_context(tc.tile_pool(name="const", bufs=1))
            psum = ctx.enter_context(tc.tile_pool(name="psum", bufs=SG,
                                                  space="PSUM"))

            hp_bounce = dram.tile([sh, OUT], BF16, tag="hp_bounce")
            hs_bounce = dram.tile([sh, OUT], BF16, tag="hs_bounce",
                                  name="hs_bounce")
            hp_fulls = [dram.tile([npad, OUT], BF16, tag=f"hp_full_{r}",
                                  addr_space="Shared", name=f"hp_full_{r}")
                        for r in range(repeat)]
            hs_fulls = [dram.tile([npad, OUT], BF16, tag=f"hs_full_{r}",
                                  addr_space="Shared", name=f"hs_full_{r}")
                        for r in range(repeat)]

            # constants
            W1_t = cpool.tile([P, kin, OUT], BF16, tag="w1")
            W23_t = cpool.tile([P, F2], BF16, tag="w23")
            b1_t = cpool.tile([P, OUT], F32, tag="b1")
            bmu_t = cpool.tile([P, 1], F32, tag="bmu")
            bls_t = cpool.tile([P, 1], F32, tag="bls")
            ns_t = cpool.tile([P, nsb], F32, tag="ns")
            nd_t = cpool.tile([P, nsb], F32, tag="nd")
            iota_t = cpool.tile([P, sb, P], BF16, tag="iota")
            idx_t = cpool.tile([P, ni16], mybir.dt.int16, tag="idx")
            dl_t = cpool.tile([P, n_inc], BF16, tag="dl")
            nc.sync.dma_start(out=W1_t[:], in_=W1_d[:].rearrange(
                "p (k o) -> p k o", k=kin))
            nc.sync.dma_start(out=W23_t[:], in_=W23_d[:])
            nc.sync.dma_start(out=b1_t[:], in_=b1_d[:])
            nc.sync.dma_start(out=bmu_t[:], in_=bmu_d[:])
            nc.sync.dma_start(out=bls_t[:], in_=bls_d[:])
            nc.sync.dma_start(out=ns_t[:], in_=ns_d[:])
            nc.sync.dma_start(out=nd_t[:], in_=nd_d[:])
            nc.sync.dma_start(out=iota_t[:], in_=iota_d[:].rearrange(
                "p (a b) -> p a b", a=sb))
            nc.sync.dma_start(out=idx_t[:], in_=idx_d[:])
            nc.sync.dma_start(out=dl_t[:], in_=dl_d[:])

            reload_inst = nc.gpsimd.load_library(library_config.mlp)

            max_call_chunks = max(ni // P for (_, _, _, ni) in calls)

            def gather_pass(table_aps, gpool, spool, chunk_sink,
                            transposed=False):
                """Shared structure of the two gather passes: per-(sg,b)
                dma_gather calls, S build per SB incidences, matmul per
                incidence into per-sblock psums.  transposed=False accumulates
                S^T @ rows (out partition = dst); transposed=True accumulates
                rows^T @ S (out partition = feature).
                chunk_sink(k_abs, ps) is called when a sblock finishes."""
                ps_of = {}
                s4 = None
                ii = 0
                ci = 0
                for g, ks in enumerate(sgs):
                    for kk in ks:
                        ps_of[kk] = psum.tile([P, OUT], F32, tag="acc",
                                              name=f"acc_{kk}")
                    for bb in range(NBUCK):
                        (gg, bb2, off, nidx) = calls[ci]
                        assert gg == g and bb2 == bb
                        ci += 1
                        gt = gpool.tile([P, max_call_chunks, OUT], BF16,
                                        tag="gt")
                        gn = P if "g128" in skip else nidx
                        gi = nc.gpsimd.dma_gather(
                            out_ap=gt[:, :gn // P, :],
                            in_ap=table_aps[bb],
                            idxs_ap=idx_t[:, off // 16:(off + gn) // 16],
                            num_idxs=gn, num_idxs_reg=gn,
                            elem_size=OUT, single_packet=False,
                            queue_num=(ci - 1) % n_qs)
                        add_dep_helper(gi.ins, reload_inst.ins, sync=False)
                        while ii < n_inc and incs[ii][1] == g \
                                and incs[ii][2] == bb:
                            kk_, g_, bb_, chg, s0, s1, st, sp = incs[ii]
                            if ii % sb == 0:
                                s4 = spool.tile([P, sb, P], BF16,
                                                tag="s4")
                                n4 = 1 if "sb1" in skip else min(
                                    sb, n_inc - ii)
                                nc.vector.tensor_tensor(
                                    out=s4[:, :n4, :],
                                    in0=iota_t[:, :n4, :],
                                    in1=dl_t[:, ii:ii + n4, None]
                                    .to_broadcast([P, n4, P]),
                                    op=mybir.AluOpType.is_equal)
                            local = chg - off // P
                            do_mm = st if "mm1" in skip else True
                            if do_mm:
                                mst = True if "mm1" in skip else st
                                msp = True if "mm1" in skip else sp
                                if transposed:
                                    nc.tensor.matmul(
                                        ps_of[kk_][:],
                                        lhsT=gt[:, local, :],
                                        rhs=s4[:, ii % sb, :],
                                        start=mst, stop=msp)
                                else:
                                    nc.tensor.matmul(
                                        ps_of[kk_][:],
                                        lhsT=s4[:, ii % sb, :],
                                        rhs=gt[:, local, :],
                                        start=mst, stop=msp)
                            ii += 1
                    for kk in ks:
                        chunk_sink(kk, ps_of[kk])
                assert ii == n_inc
            def one_iter(hp_full, hs_full):
                # ------------- P1: hp = (feat @ W1) * ns -------------
                with tc.tile_pool(name="featT", bufs=1) as fpool, \
                     tc.tile_pool(name="p1work", bufs=4) as wpool:
                    fT = fpool.tile([P, kin, sh], BF16, tag="fT", name="fT")
                    STRIP = 8
                    for s0 in range(0, nsb, STRIP):
                        s1 = min(s0 + STRIP, nsb)
                        # load only this strip's feature columns so the first
                        # matmuls (and thus AG1) start ~30us earlier
                        nc.sync.dma_start(
                            out=fT[:, :, s0 * P:s1 * P],
                            in_=featT_d[:].rearrange(
                                "p (k s) -> p k s", k=kin)[:, :, s0 * P:s1 * P])
                        strip = wpool.tile([P, STRIP, OUT], BF16,
                                           tag="hpstrip", name="hpstrip")
                        for rt in range(s0, s1):
                            ps = psum.tile([P, OUT], F32, tag="acc",
                                           name="p1ps")
                            for kc in range(kin):
                                nc.tensor.matmul(
                                    ps[:],
                                    lhsT=fT[:, kc, rt * P:(rt + 1) * P],
                                    rhs=W1_t[:, kc, :],
                                    start=(kc == 0), stop=(kc == kin - 1))
                            nc.vector.tensor_scalar_mul(
                                strip[:, rt - s0, :], ps[:],
                                ns_t[:, rt:rt + 1])
                        nc.sync.dma_start(
                            out=hp_bounce[:].rearrange("(t p) o -> p t o",
                                                       p=P)[:, s0:s1, :],
                            in_=strip[:, :s1 - s0, :])

                if "ag" not in skip:
                    nc.gpsimd.collective_compute(
                        "AllGather", mybir.AluOpType.bypass,
                        ins=[hp_bounce.opt()], outs=[hp_full.opt()],
                        replica_groups=replica)

                # ------------- P2: gather+aggregate layer 1 -> hs ------
                with tc.tile_pool(name="g1", bufs=4) as gpool, \
                     tc.tile_pool(name="s1", bufs=4) as spool, \
                     tc.tile_pool(name="h1", bufs=4) as hpool, \
                     tc.tile_pool(name="hss", bufs=2) as hsspool:

                    hs_strips = {}

                    def sink1(kk, ps):
                        g = kk // SG
                        j = kk % SG
                        if j == 0:
                            hs_strips[g] = hsspool.tile(
                                [P, SG, P], BF16, tag="hss", name=f"hss_{g}")
                        t1 = hpool.tile([P, OUT], F32, tag="t1", name="t1")
                        nc.vector.tensor_scalar_mul(t1[:], ps[:],
                                                    nd_t[:, kk:kk + 1])
                        if not zero_b1:
                            nc.vector.tensor_tensor(out=t1[:], in0=t1[:],
                                                    in1=b1_t[:],
                                                    op=mybir.AluOpType.add)
                        hrow = hpool.tile([P, OUT], BF16, tag="hrow",
                                          name="hrow")
                        nc.scalar.activation(
                            hrow[:], t1[:],
                            mybir.ActivationFunctionType.Relu)
                        nc.vector.tensor_scalar_mul(hs_strips[g][:, j, :],
                                                    hrow[:],
                                                    ns_t[:, kk:kk + 1])
                        last = (kk == nsb - 1)
                        if j == SG - 1 or last:
                            n = j + 1
                            k0 = kk - j
                            nc.sync.dma_start(
                                out=hs_bounce[:].rearrange(
                                    "(t p) o -> p t o",
                                    p=P)[:, k0:k0 + n, :],
                                in_=hs_strips[g][:, :n, :])

                    if "gather" not in skip:
                        gather_pass([hp_full[bb * brows:(bb + 1) * brows, :]
                                     for bb in range(NBUCK)], gpool,
                                    spool, sink1)

                if "ag" not in skip:
                    nc.gpsimd.collective_compute(
                        "AllGather", mybir.AluOpType.bypass,
                        ins=[hs_bounce.opt()], outs=[hs_full.opt()],
                        replica_groups=replica)

                # ------------- P4: gather+aggregate layers 2/3 ----------
                with tc.tile_pool(name="g2", bufs=4) as gpool2, \
                     tc.tile_pool(name="s2", bufs=4) as spool2, \
                     tc.tile_pool(name="e2", bufs=6) as epool, \
                     tc.tile_pool(name="nzp", bufs=4) as npool, \
                     tc.tile_pool(name="outs", bufs=2) as outpool:

                    noiseT_g = {}
                    ndb_g = {}

                    def load_group(g):
                        if g >= len(sgs):
                            return
                        k0 = sgs[g][0]
                        n = len(sgs[g])
                        noiseT_g[g] = npool.tile([P, SG, P], F32,
                                                 tag="noiseT",
                                                 name=f"noiseT_{g}")
                        nc.sync.dma_start(
                            out=noiseT_g[g][:, :n, :],
                            in_=noiseT_d[:].rearrange(
                                "p (k d) -> p k d",
                                k=nsb)[:, k0:k0 + n, :])
                        ndb_g[g] = npool.tile([P, SG, P], F32, tag="ndb",
                                              name=f"ndb_{g}")
                        nc.sync.dma_start(
                            out=ndb_g[g][:, :n, :],
                            in_=ndb_d[:].rearrange(
                                "p (k d) -> p k d",
                                k=nsb)[:, k0:k0 + n, :])

                    load_group(0)

                    out_strips = {}

                    def sink2(kk, ps):
                        g = kk // SG
                        j = kk % SG
                        if j == 0:
                            out_strips[g] = outpool.tile(
                                [P, SG, P], F32, tag="outs",
                                name=f"os_{g}")
                            load_group(g + 1)
                        # zn = z2^T * nd (free-dim broadcast), cast to bf16
                        znb = epool.tile([P, P], BF16, tag="znb", name="znb")
                        nc.vector.tensor_tensor(out=znb[:], in0=ps[:],
                                                in1=ndb_g[g][:, j, :],
                                                op=mybir.AluOpType.mult)
                        ps2 = psum.tile([P, 2, P], F32, tag="acc",
                                        name="ps2")
                        nc.tensor.matmul(ps2[:, 0, :], lhsT=W23_t[:, 0:OUT],
                                         rhs=znb[:], start=True, stop=True)
                        nc.tensor.matmul(ps2[:, 1, :], lhsT=W23_t[:, OUT:F2],
                                         rhs=znb[:], start=True, stop=True)
                        # sig = exp(ls^T + bls) * noise^T
                        sig = epool.tile([P, P], F32, tag="sig", name="sig")
                        nc.scalar.activation(
                            sig[:], ps2[:, 1, :],
                            mybir.ActivationFunctionType.Exp,
                            bias=(0.0 if zero_b23 else bls_t[:, 0:1]))
                        nc.vector.tensor_tensor(out=sig[:], in0=sig[:],
                                                in1=noiseT_g[g][:, j, :],
                                                op=mybir.AluOpType.mult)
                        # y^T = mu^T + sig  (mu bias is zero in this problem;
                        # fall back to an extra add if not)
                        if zero_b23:
                            nc.vector.tensor_tensor(
                                out=out_strips[g][:, j, :],
                                in0=ps2[:, 0, :], in1=sig[:],
                                op=mybir.AluOpType.add)
                        else:
                            tmu = epool.tile([P, P], F32, tag="tmu",
                                             name="tmu")
                            nc.vector.tensor_scalar_add(tmu[:], ps2[:, 0, :],
                                                        bmu_t[:, 0:1])
                            nc.vector.tensor_tensor(
                                out=out_strips[g][:, j, :],
                                in0=tmu[:], in1=sig[:],
                                op=mybir.AluOpType.add)
                        last = (kk == nsb - 1)
                        if j == SG - 1 or last:
                            n = j + 1
                            k0 = kk - j
                            nc.sync.dma_start(
                                out=y_d[:, k0 * P:(kk + 1) * P],
                                in_=out_strips[g][:, :n, :].rearrange(
                                    "p a b -> p (a b)"))

                    if "gather" not in skip:
                        gather_pass([hs_full[bb * brows:(bb + 1) * brows, :]
                                     for bb in range(NBUCK)], gpool2,
                                    spool2, sink2, transposed=True)

            for _rep in range(repeat):
                one_iter(hp_fulls[_rep], hs_fulls[_rep])

    nc.compile()
    return nc, in_maps, N


_CACHE = {}


def _run(feat, edges, W1, b1, W_mu, b_mu, W_ls, b_ls, noise, trace=False):
    import hashlib
    h = hashlib.sha1()
    for a in (edges, feat, W1, b1, W_mu, b_mu, W_ls, b_ls, noise):
        h.update(np.ascontiguousarray(a).tobytes())
    key = h.hexdigest()
    if key in _CACHE:
        nc, in_maps, N = _CACHE[key]
    else:
        nc, in_maps, N = _build(feat, edges, W1, b1, W_mu, b_mu, W_ls, b_ls,
                                noise)
        _CACHE[key] = (nc, in_maps, N)
    res = run_bass_kernel_spmd(nc, in_maps, core_ids=list(range(NC)),
                               trace=trace)
    # y is [128 feat, sh] per core; un-transpose and concatenate shards
    out = np.concatenate([res.results[c]["y"].T for c in range(NC)], axis=0)
    return out[:N], res


def kernel(feat, edges, W1, b1, W_mu, b_mu, W_ls, b_ls, noise):
    out, _ = _run(np.asarray(feat), np.asarray(edges), np.asarray(W1),
                  np.asarray(b1), np.asarray(W_mu), np.asarray(b_mu),
                  np.asarray(W_ls), np.asarray(b_ls), np.asarray(noise))
    return out
